# revision 3
# baseline (speedup 1.0000x reference)
"""Trainium2 Bass kernel for nn_ModelNew_78847009620052 (dense_mlp).

Computes, for x [4096, 8192] and weight [8192, 8192]:
    out[b, 0] = 0.75 * sum_i x[b, i] * (sum_j weight[j, i])
(which equals 1.5 * sum(x @ W.T / 2, axis=1, keepdims=True)).

Sharding: column-shard the contraction dim IN=8192 into 8 chunks of 1024.
Core d receives x[:, d*1024:(d+1)*1024] and weight[:, d*1024:(d+1)*1024],
produces a partial [4096, 1]; host sums the 8 partials.

Per-core device algorithm (memory-bound: 48MB of input per core):
  Phase 1: stream 64 weight row-tiles [128, 1024]; pair-wise add on VectorE
           (32 adds), then accumulate the 32 summed pairs on TensorE via
           matmul with an all-ones [128, 128] stationary operand - this both
           reduces over the partition (row) axis and broadcasts the column
           sums to all 128 output partitions in one op. PSUM [128, 1024].
  Phase 2: stream 32 x row-tiles [128, 1024]; multiply against the broadcast
           column sums on VectorE (in place), then reduce along the free dim
           on ScalarE via activation(Copy, accum_out=...) with the 0.75 scale
           folded in. Results collect in an SBUF [128, 32] tile, written out
           with a rearranged access pattern to [4096, 1].

(tensor_tensor_reduce would fuse phase 2 into one VectorE op, but that
opcode crashes the device on this HW/NRT path - validated by bisection.)
"""

import numpy as np

B, IN, HID = 4096, 8192, 8192
N_CORES = 8
CHUNK = IN // N_CORES          # 1024 columns per core
SCALE = 1.5 / 2.0              # 0.75
P = 128                        # partitions
W_TILES = HID // P             # 64 weight row-tiles per core
X_TILES = B // P               # 32 x row-tiles per core

_compiled_nc = None


def _build_nc():
    import concourse.bass as bass
    import concourse.tile as tile
    from concourse import bacc, mybir

    f32 = mybir.dt.float32
    nc = bacc.Bacc(
        "TRN2",
        target_bir_lowering=False,
        debug=False,
        num_devices=N_CORES,
    )

    x_d = nc.dram_tensor("x", [B, CHUNK], f32, kind="ExternalInput")
    w_d = nc.dram_tensor("w", [HID, CHUNK], f32, kind="ExternalInput")
    out_d = nc.dram_tensor("out", [B, 1], f32, kind="ExternalOutput")

    with tile.TileContext(nc) as tc:
        with (
            tc.tile_pool(name="wpool", bufs=6) as wpool,
            tc.tile_pool(name="xpool", bufs=6) as xpool,
            tc.tile_pool(name="const", bufs=1) as const,
            tc.tile_pool(name="psum", bufs=1, space="PSUM") as psum_pool,
        ):
            ones = const.tile([P, P], f32)
            nc.gpsimd.memset(ones[:], 1.0)

            # Phase 1: column sums of w chunk, reduced over all 8192 rows.
            psum_bc = psum_pool.tile([P, CHUNK], f32)  # 2 banks
            for j in range(W_TILES // 2):
                wa = wpool.tile([P, CHUNK], f32, tag="wtile")
                nc.sync.dma_start(wa[:], w_d[(2 * j) * P : (2 * j + 1) * P, :])
                wb = wpool.tile([P, CHUNK], f32, tag="wtile")
                nc.sync.dma_start(wb[:], w_d[(2 * j + 1) * P : (2 * j + 2) * P, :])
                nc.vector.tensor_add(wa[:], wa[:], wb[:])
                for h in range(2):
                    nc.tensor.matmul(
                        psum_bc[:, h * 512 : (h + 1) * 512],
                        ones[:],
                        wa[:, h * 512 : (h + 1) * 512],
                        start=(j == 0),
                        stop=(j == W_TILES // 2 - 1),
                    )

            # Broadcast column sums now live in every PSUM partition; move to
            # SBUF on ScalarE so VectorE stays free for phase 2.
            w_bcast = const.tile([P, CHUNK], f32)
            nc.scalar.copy(w_bcast[:], psum_bc[:])

            # Phase 2: multiply + reduce of x tiles against w_bcast.
            s_sbuf = const.tile([P, X_TILES], f32)
            scratch = const.tile([P, CHUNK], f32)
            for i in range(X_TILES):
                xt = xpool.tile([P, CHUNK], f32, tag="xtile")
                nc.sync.dma_start(xt[:], x_d[i * P : (i + 1) * P, :])
                nc.vector.tensor_mul(xt[:], xt[:], w_bcast[:])
                nc.scalar.activation(
                    scratch[:],
                    xt[:],
                    mybir.ActivationFunctionType.Copy,
                    bias=0.0,
                    scale=SCALE,
                    accum_out=s_sbuf[:, i : i + 1],
                )

            # out[n*128 + p, 0] = s_sbuf[p, n]
            out_ap = out_d[:].rearrange("(n p) o -> p (n o)", p=P)
            nc.sync.dma_start(out_ap, s_sbuf[:])

    nc.compile()
    return nc


def _get_nc():
    global _compiled_nc
    if _compiled_nc is None:
        _compiled_nc = _build_nc()
    return _compiled_nc


def kernel(x: np.ndarray, weight: np.ndarray) -> np.ndarray:
    from concourse.bass_utils import run_bass_kernel_spmd

    x = np.asarray(x, dtype=np.float32)
    weight = np.asarray(weight, dtype=np.float32)
    assert x.shape == (B, IN) and weight.shape == (HID, IN)

    nc = _get_nc()
    in_maps = [
        {
            "x": np.ascontiguousarray(x[:, d * CHUNK : (d + 1) * CHUNK]),
            "w": np.ascontiguousarray(weight[:, d * CHUNK : (d + 1) * CHUNK]),
        }
        for d in range(N_CORES)
    ]
    res = run_bass_kernel_spmd(nc, in_maps, core_ids=list(range(N_CORES)))
    acc = np.zeros((B, 1), dtype=np.float64)
    for d in range(N_CORES):
        acc += res.results[d]["out"].astype(np.float64)
    return acc.astype(np.float32)


# revision 5
# speedup vs baseline: 1.0162x; 1.0162x over previous
"""Trainium2 Bass kernel for nn_ModelNew_78847009620052 (dense_mlp).

Computes, for x [4096, 8192] and weight [8192, 8192]:
    out[b, 0] = 0.75 * sum_i x[b, i] * (sum_j weight[j, i])
(which equals 1.5 * sum(x @ W.T / 2, axis=1, keepdims=True)).

Sharding: column-shard the contraction dim IN=8192 into 8 chunks of 1024.
Core d receives x[:, d*1024:(d+1)*1024] and weight[:, d*1024:(d+1)*1024],
produces a partial [4096, 1]; host sums the 8 partials.

Per-core device algorithm (memory-bound: 48MB of input per core):
  Phase 1: stream 64 weight row-tiles [128, 1024]; accumulate groups of 4
           on VectorE (3 adds per group), then accumulate the 16 group sums
           on TensorE via matmul with an all-ones [128, 128] stationary
           operand - this both reduces over the partition (row) axis and
           broadcasts the column sums to all 128 output partitions in one
           op. PSUM [128, 1024]. (fp32 matmul runs at 4 cyc/row and each
           matmul re-loads the ones weights, so PE work must be kept well
           under the weight-DMA window - hence the DVE pre-accumulation.)
  Phase 2: stream 32 x row-tiles [128, 1024]; multiply against the broadcast
           column sums on VectorE (in place), then reduce along the free dim
           on ScalarE via activation(Copy, accum_out=...) with the 0.75 scale
           folded in. Results collect in an SBUF [128, 32] tile, written out
           with a rearranged access pattern to [4096, 1].

(tensor_tensor_reduce would fuse phase 2 into one VectorE op, but that
opcode crashes the device on this HW/NRT path - validated by bisection.)
"""

import numpy as np

B, IN, HID = 4096, 8192, 8192
N_CORES = 8
CHUNK = IN // N_CORES          # 1024 columns per core
SCALE = 1.5 / 2.0              # 0.75
P = 128                        # partitions
W_TILES = HID // P             # 64 weight row-tiles per core
X_TILES = B // P               # 32 x row-tiles per core

_compiled_nc = None


def _build_nc():
    import concourse.bass as bass
    import concourse.tile as tile
    from concourse import bacc, mybir

    f32 = mybir.dt.float32
    nc = bacc.Bacc(
        "TRN2",
        target_bir_lowering=False,
        debug=False,
        num_devices=N_CORES,
    )

    x_d = nc.dram_tensor("x", [B, CHUNK], f32, kind="ExternalInput")
    w_d = nc.dram_tensor("w", [HID, CHUNK], f32, kind="ExternalInput")
    out_d = nc.dram_tensor("out", [B, 1], f32, kind="ExternalOutput")

    with tile.TileContext(nc) as tc:
        with (
            tc.tile_pool(name="wpool", bufs=10) as wpool,
            tc.tile_pool(name="xpool", bufs=6) as xpool,
            tc.tile_pool(name="const", bufs=1) as const,
            tc.tile_pool(name="psum", bufs=1, space="PSUM") as psum_pool,
        ):
            ones = const.tile([P, P], f32)
            nc.vector.memset(ones[:], 1.0)

            # Phase 1: column sums of w chunk, reduced over all 8192 rows.
            GROUP = 4
            n_groups = W_TILES // GROUP  # 16
            psum_bc = psum_pool.tile([P, CHUNK], f32)  # 2 banks
            for j in range(n_groups):
                wts = []
                for k in range(GROUP):
                    wt = wpool.tile([P, CHUNK], f32, tag="wtile")
                    nc.sync.dma_start(
                        wt[:], w_d[(GROUP * j + k) * P : (GROUP * j + k + 1) * P, :]
                    )
                    wts.append(wt)
                # acc = (w0 + w1) + (w2 + w3), in place
                nc.vector.tensor_add(wts[0][:], wts[0][:], wts[1][:])
                nc.vector.tensor_add(wts[2][:], wts[2][:], wts[3][:])
                nc.vector.tensor_add(wts[0][:], wts[0][:], wts[2][:])
                for h in range(2):
                    nc.tensor.matmul(
                        psum_bc[:, h * 512 : (h + 1) * 512],
                        ones[:],
                        wts[0][:, h * 512 : (h + 1) * 512],
                        start=(j == 0),
                        stop=(j == n_groups - 1),
                    )

            # Broadcast column sums now live in every PSUM partition; move to
            # SBUF on ScalarE so VectorE stays free for phase 2.
            w_bcast = const.tile([P, CHUNK], f32)
            nc.scalar.copy(w_bcast[:], psum_bc[:])

            # Phase 2: multiply + reduce of x tiles against w_bcast.
            s_sbuf = const.tile([P, X_TILES], f32)
            scratch = const.tile([P, CHUNK], f32)
            for i in range(X_TILES):
                xt = xpool.tile([P, CHUNK], f32, tag="xtile")
                nc.sync.dma_start(xt[:], x_d[i * P : (i + 1) * P, :])
                nc.vector.tensor_mul(xt[:], xt[:], w_bcast[:])
                nc.scalar.activation(
                    scratch[:],
                    xt[:],
                    mybir.ActivationFunctionType.Copy,
                    bias=0.0,
                    scale=SCALE,
                    accum_out=s_sbuf[:, i : i + 1],
                )

            # out[n*128 + p, 0] = s_sbuf[p, n]
            out_ap = out_d[:].rearrange("(n p) o -> p (n o)", p=P)
            nc.sync.dma_start(out_ap, s_sbuf[:])

    nc.compile()
    return nc


def _get_nc():
    global _compiled_nc
    if _compiled_nc is None:
        _compiled_nc = _build_nc()
    return _compiled_nc


def kernel(x: np.ndarray, weight: np.ndarray) -> np.ndarray:
    from concourse.bass_utils import run_bass_kernel_spmd

    x = np.asarray(x, dtype=np.float32)
    weight = np.asarray(weight, dtype=np.float32)
    assert x.shape == (B, IN) and weight.shape == (HID, IN)

    nc = _get_nc()
    in_maps = [
        {
            "x": np.ascontiguousarray(x[:, d * CHUNK : (d + 1) * CHUNK]),
            "w": np.ascontiguousarray(weight[:, d * CHUNK : (d + 1) * CHUNK]),
        }
        for d in range(N_CORES)
    ]
    res = run_bass_kernel_spmd(nc, in_maps, core_ids=list(range(N_CORES)))
    acc = np.zeros((B, 1), dtype=np.float64)
    for d in range(N_CORES):
        acc += res.results[d]["out"].astype(np.float64)
    return acc.astype(np.float32)


# revision 7
# speedup vs baseline: 1.1010x; 1.0834x over previous
"""Trainium2 Bass kernel for nn_ModelNew_78847009620052 (dense_mlp).

Computes, for x [4096, 8192] and weight [8192, 8192]:
    out[b, 0] = 0.75 * sum_i x[b, i] * (sum_j weight[j, i])
(which equals 1.5 * sum(x @ W.T / 2, axis=1, keepdims=True)).

Sharding: column-shard the contraction dim IN=8192 into 8 chunks of 1024.
Core d receives x[:, d*1024:(d+1)*1024] and weight[:, d*1024:(d+1)*1024],
produces a partial [4096, 1]; host sums the 8 partials.

Per-core device algorithm (memory-bound: 48MB of input per core):
  Phase 1: stream 64 weight row-tiles [128, 1024]; accumulate groups of 4
           on VectorE (3 adds per group), then accumulate the 16 group sums
           on TensorE via matmul with an all-ones [128, 128] stationary
           operand - this both reduces over the partition (row) axis and
           broadcasts the column sums to all 128 output partitions in one
           op. PSUM [128, 1024]. (fp32 matmul runs at 4 cyc/row and each
           matmul re-loads the ones weights, so PE work must be kept well
           under the weight-DMA window - hence the DVE pre-accumulation.)
  Phase 2: stream 32 x row-tiles [128, 1024]; multiply against the broadcast
           column sums on VectorE (in place), then reduce along the free dim
           on ScalarE via activation(Copy, accum_out=...) with the 0.75 scale
           folded in. Results collect in an SBUF [128, 32] tile, written out
           with a rearranged access pattern to [4096, 1].

(tensor_tensor_reduce would fuse phase 2 into one VectorE op, but that
opcode crashes the device on this HW/NRT path - validated by bisection.)
"""

import numpy as np

B, IN, HID = 4096, 8192, 8192
N_CORES = 8
CHUNK = IN // N_CORES          # 1024 columns per core
SCALE = 1.5 / 2.0              # 0.75
P = 128                        # partitions
W_TILES = HID // P             # 64 weight row-tiles per core
X_TILES = B // P               # 32 x row-tiles per core

_compiled_nc = None


def _build_nc():
    import concourse.bass as bass
    import concourse.tile as tile
    from concourse import bacc, mybir

    f32 = mybir.dt.float32
    nc = bacc.Bacc(
        "TRN2",
        target_bir_lowering=False,
        debug=False,
        num_devices=N_CORES,
    )

    x_d = nc.dram_tensor("x", [B, CHUNK], f32, kind="ExternalInput")
    w_d = nc.dram_tensor("w", [HID, CHUNK], f32, kind="ExternalInput")
    out_d = nc.dram_tensor("out", [B, 1], f32, kind="ExternalOutput")

    with tile.TileContext(nc) as tc:
        with (
            tc.tile_pool(name="wpool", bufs=18) as wpool,
            tc.tile_pool(name="xpool", bufs=12) as xpool,
            tc.tile_pool(name="const", bufs=1) as const,
            tc.tile_pool(name="psum", bufs=1, space="PSUM") as psum_pool,
        ):
            from concourse.masks import make_identity

            ones = const.tile([P, P], f32)
            nc.vector.memset(ones[:], 1.0)
            identity = const.tile([P, P], f32)
            make_identity(nc, identity)

            # Phase 1: column sums of w chunk, reduced over all 8192 rows.
            GROUP = 8
            n_groups = W_TILES // GROUP  # 8
            psum_bc = psum_pool.tile([P, CHUNK], f32, tag="psum_bc")  # 2 banks
            for j in range(n_groups):
                wts = []
                for k in range(GROUP):
                    wt = wpool.tile([P, CHUNK], f32, tag="wtile")
                    nc.sync.dma_start(
                        wt[:], w_d[(GROUP * j + k) * P : (GROUP * j + k + 1) * P, :]
                    )
                    wts.append(wt)
                # tree-reduce the 8 tiles in place on VectorE (7 adds)
                for s in (1, 2, 4):
                    for k in range(0, GROUP, 2 * s):
                        nc.vector.tensor_add(
                            wts[k][:], wts[k][:], wts[k + s][:]
                        )
                for h in range(2):
                    nc.tensor.matmul(
                        psum_bc[:, h * 512 : (h + 1) * 512],
                        ones[:],
                        wts[0][:, h * 512 : (h + 1) * 512],
                        start=(j == 0),
                        stop=(j == n_groups - 1),
                    )

            # Broadcast column sums now live in every PSUM partition; move to
            # SBUF on ScalarE so VectorE stays free for phase 2.
            w_bcast = const.tile([P, CHUNK], f32)
            nc.scalar.copy(w_bcast[:], psum_bc[:])

            # Phase 2: multiply + reduce of x tiles against w_bcast.
            s_sbuf = const.tile([P, X_TILES], f32)
            scratch = const.tile([P, CHUNK], f32)
            for i in range(X_TILES):
                xt = xpool.tile([P, CHUNK], f32, tag="xtile")
                nc.sync.dma_start(xt[:], x_d[i * P : (i + 1) * P, :])
                nc.vector.tensor_mul(xt[:], xt[:], w_bcast[:])
                nc.scalar.activation(
                    scratch[:],
                    xt[:],
                    mybir.ActivationFunctionType.Copy,
                    bias=0.0,
                    scale=SCALE,
                    accum_out=s_sbuf[:, i : i + 1],
                )

            # Transpose s_sbuf [128, 32] -> [32, 128] on TensorE so the store
            # is contiguous 512B runs in DRAM (a [128, 32]-layout store would
            # shatter into 4096 4-byte DMA packets - measured 16us).
            psum_t = psum_pool.tile([X_TILES, P], f32, tag="psum_t")
            nc.tensor.transpose(psum_t[:], s_sbuf[:], identity[:])
            sT = const.tile([X_TILES, P], f32)
            nc.scalar.copy(sT[:], psum_t[:])
            # out[n*128 + p, 0] = sT[n, p]
            out_ap = out_d[:].rearrange("(n p) o -> n (p o)", p=P)
            nc.sync.dma_start(out_ap, sT[:])

    nc.compile()
    return nc


def _get_nc():
    global _compiled_nc
    if _compiled_nc is None:
        _compiled_nc = _build_nc()
    return _compiled_nc


def kernel(x: np.ndarray, weight: np.ndarray) -> np.ndarray:
    from concourse.bass_utils import run_bass_kernel_spmd

    x = np.asarray(x, dtype=np.float32)
    weight = np.asarray(weight, dtype=np.float32)
    assert x.shape == (B, IN) and weight.shape == (HID, IN)

    nc = _get_nc()
    in_maps = [
        {
            "x": np.ascontiguousarray(x[:, d * CHUNK : (d + 1) * CHUNK]),
            "w": np.ascontiguousarray(weight[:, d * CHUNK : (d + 1) * CHUNK]),
        }
        for d in range(N_CORES)
    ]
    res = run_bass_kernel_spmd(nc, in_maps, core_ids=list(range(N_CORES)))
    acc = np.zeros((B, 1), dtype=np.float64)
    for d in range(N_CORES):
        acc += res.results[d]["out"].astype(np.float64)
    return acc.astype(np.float32)
